# revision 3
# baseline (speedup 1.0000x reference)
"""ChildSumTreeLSTM on 8 Trainium2 NeuronCores — feature-sharded.

Sharding: core c owns feature rows [128c, 128c+128) of EVERYTHING (h, c_state,
all gate outputs, z = Wrel @ hsum). All contractions over the full 1024-wide
feature/input dim happen locally against replicated activations:
  - x^T is fully SBUF-resident -> xi/xf slices local, no exchange
  - h is AllGathered after each wave (the only exchanged state)
  - z (rel-transformed hsum) is AllGathered once per wave
  - c never leaves its core (f (.) c is feature-elementwise)
  - all Wrel slices are SBUF-resident (1/8 rows of each relation matrix),
    so relation matmuls need no per-wave weight streaming, unlike the old
    rel-sharded design (22 MB of DMA per run)

Per internal wave: strided child gather + hsum reduce (DVE) -> per-relation
matmul batches into PSUM, masked accumulation on DVE -> z AllGather ->
iou/f gate matmuls + activations -> h AllGather. Waves with a single
relation instead use a host-fused M = iouh_w @ W_r (skips the z exchange).

This cuts the baseline's 10 collectives to 8 smaller ones and its ~22MB of
weight streaming to a one-time ~12MB preload that overlaps the xi/leaf phase.
"""

import sys

sys.path.insert(0, "/opt/trn_rl_repo")

import numpy as np
import ml_dtypes

import concourse.bass as bass
import concourse.mybir as mybir
import concourse.tile as tile
from concourse.bass_utils import run_bass_kernel_spmd
from concourse.vector_clock import ScopedClock, VectorClock

BF16 = mybir.dt.bfloat16
F32 = mybir.dt.float32
F8 = mybir.dt.float8e4
NCORES = 8
P = 128
FUSE_MAX_RELS = 6   # waves with <= this many rels use fused M_r (no z round)


# This walrus build rejects >1 sem wait per instruction at the Tile exit
# drain; split the aggregated drain into one drain per proc.
def _split_drain_and_barrier(self, tick_clock, wait_clock):
    gc = tick_clock.global_clock
    n = len(gc)
    nonzero = [i for i in range(n) if gc[i] > 0]
    for j in nonzero:
        vec = VectorClock([gc[i] if i == j else 0 for i in range(n)])
        d = self.nc.sync.drain()
        wait_clock.add_sem_waits(d.ins, ScopedClock({None: vec}))
    if not nonzero:
        d = self.nc.sync.drain()
        wait_clock.add_sem_waits(d.ins, ScopedClock({None: gc.copy()}))
    self.nc.all_engine_barrier()
    assert self.sems is not None
    popped = self.nc._tile_sem_poison_stack.pop()
    assert popped is self._sem_poison
    self.nc.clear_and_free_semaphores(list(self.sems.allocated().values()))
    self.nc.all_engine_barrier()


tile.TileContext._drain_and_barrier = _split_drain_and_barrier


def _split_multi_waits(nc, limit=1):
    """Walrus here allows only one sem wait per instruction; hoist extras
    onto same-engine NOPs inserted right before the instruction."""
    for bb in nc.main_func.blocks:
        new_list = []
        for ins in bb.instructions:
            si = getattr(ins, "sync_info", None)
            if si is not None and si.on_wait and len(si.on_wait) > limit:
                waits = list(si.on_wait)
                for w in waits[:-limit]:
                    nop = mybir.InstNoOp(
                        name=nc.get_next_instruction_name(),
                        sync_info=mybir.SyncInfo(on_wait=[w], on_update=[]),
                        bass_nofuse=True,
                        engine=ins.engine,
                    )
                    nc.register_instruction(nop, overwrite=True)
                    new_list.append(nop)
                si.on_wait = waits[-limit:]
            new_list.append(ins)
        bb.instructions[:] = new_list


def _bf16(a):
    return np.ascontiguousarray(a.astype(ml_dtypes.bfloat16))


def _plan(child_idx, rel_ids, Wrel):
    """Waves, column order (wave-major, heap order within wave), child runs,
    per-wave relation lists."""
    N, K = child_idx.shape
    eff_children = []
    wave = np.zeros(N, np.int32)
    for i in range(N):
        cs = [int(c) for c in child_idx[i] if 0 <= c < i]
        eff_children.append(cs)
        wave[i] = 1 + max((wave[c] for c in cs), default=-1)
    nwaves = int(wave.max()) + 1
    order = sorted(range(N), key=lambda i: (wave[i], -i))
    col_of = np.empty(N, np.int64)
    for j, node in enumerate(order):
        col_of[node] = j
    waves = []
    j = 0
    for w in range(nwaves):
        cnt = int((wave == w).sum())
        waves.append((j, j + cnt))
        j += cnt

    ident = set()
    eye = np.eye(Wrel.shape[1], dtype=Wrel.dtype)
    for r in set(int(r_) for r_ in rel_ids):
        if np.array_equal(Wrel[r], eye):
            ident.add(r)

    # per internal wave: if ALL rels are identity, z == hsum (no matmul).
    wave_rels = []
    for w in range(1, nwaves):
        p0, p1 = waves[w]
        rels_all = sorted(set(int(rel_ids[order[j]]) for j in range(p0, p1)))
        if set(rels_all) <= ident:
            wave_rels.append([])
        else:
            wave_rels.append(rels_all)

    ZCOL = N
    child_col = np.full((N, K), ZCOL, np.int64)
    for i in range(N):
        for kk, c in enumerate(eff_children[i]):
            child_col[i, kk] = col_of[c]
    wave_runs = []
    for w in range(1, nwaves):
        p0, p1 = waves[w]
        seq = []
        for j in range(p0, p1):
            seq.extend(child_col[order[j]])
        runs = []
        i0 = 0
        while i0 < len(seq):
            i1 = i0 + 1
            while i1 < len(seq) and seq[i1] == seq[i1 - 1] + 1:
                i1 += 1
            runs.append((i0, int(seq[i0]), i1 - i0))
            i0 = i1
        wave_runs.append(runs)
    return dict(order=order, col_of=col_of, waves=waves, nwaves=nwaves,
                wave_rels=wave_rels, wave_runs=wave_runs, ident=ident,
                child_col=child_col)


def kernel(**inputs):
    x = np.asarray(inputs["x"], np.float32)
    Wrel = np.asarray(inputs["Wrel"], np.float32)
    ioux_w = np.asarray(inputs["ioux_w"], np.float32)
    ioux_b = np.asarray(inputs["ioux_b"], np.float32)
    iouh_w = np.asarray(inputs["iouh_w"], np.float32)
    iouh_b = np.asarray(inputs["iouh_b"], np.float32)
    fx_w = np.asarray(inputs["fx_w"], np.float32)
    fx_b = np.asarray(inputs["fx_b"], np.float32)
    fh_w = np.asarray(inputs["fh_w"], np.float32)
    fh_b = np.asarray(inputs["fh_b"], np.float32)
    child_idx = np.asarray(inputs["child_idx"], np.int32)
    rel_ids = np.asarray(inputs["rel_ids"], np.int32)

    N, IN_DIM = x.shape
    MEM = fh_w.shape[0]
    KC = MEM // P
    KX = IN_DIM // P
    K = child_idx.shape[1]
    NPAD = N + 1
    assert KC == NCORES and KX == NCORES

    plan = _plan(child_idx, rel_ids, Wrel)
    order, col_of, waves = plan["order"], plan["col_of"], plan["waves"]
    nwaves, wave_rels, wave_runs = (plan["nwaves"], plan["wave_rels"],
                                    plan["wave_runs"])

    fused = [bool(rels) and len(rels) <= FUSE_MAX_RELS for rels in wave_rels]

    # unfused rel slots, ordered by first use
    used = []
    for wi, rels in enumerate(wave_rels):
        if fused[wi]:
            continue
        for r in rels:
            if r not in used:
                used.append(r)
    slot_of = {r: i for i, r in enumerate(used)}
    NR = max(len(used), 1)

    # fused-wave M_r = iouh_w @ Wrel[r] slots
    fused_rels = []
    for wi, rels in enumerate(wave_rels):
        if fused[wi]:
            for r in rels:
                if r not in fused_rels:
                    fused_rels.append(r)
    fslot_of = {r: i for i, r in enumerate(fused_rels)}
    NF = max(len(fused_rels), 1)

    NMAX = max((waves[w][1] - waves[w][0]) for w in range(1, nwaves))
    n_leaf = waves[0][1] - waves[0][0]

    # per-(wave, rel) column runs: relations own disjoint column sets, so
    # the masked accumulate is just column-slice copies out of PSUM
    rel_runs = {}
    for wi, rels in enumerate(wave_rels):
        if fused[wi]:
            continue
        p0, p1 = waves[wi + 1]
        for r in rels:
            cols = [t for t in range(p1 - p0)
                    if int(rel_ids[order[p0 + t]]) == r]
            runs = []
            for t in cols:
                if runs and t == runs[-1][0] + runs[-1][1]:
                    runs[-1][1] += 1
                else:
                    runs.append([t, 1])
            rel_runs[(wi, r)] = runs

    # ---- per-core host data -------------------------------------------------
    xT = np.ascontiguousarray(x[order].T)
    xt_h = np.zeros((KX, P, N), ml_dtypes.bfloat16)
    for k in range(KX):
        xt_h[k] = _bf16(xT[k * P:(k + 1) * P])

    Mfused = {r: iouh_w @ Wrel[r] for r in fused_rels}

    iouxs_h, fxs_h, iouhs_h, fhs_h, wrel_h, mf_h = [], [], [], [], [], []
    b_xi = [np.zeros((3, P), np.float32) for _ in range(NCORES)]
    b_iou = [np.zeros((3, P), np.float32) for _ in range(NCORES)]
    b_xf = [np.zeros((P,), np.float32) for _ in range(NCORES)]
    b_fh = [np.zeros((P,), np.float32) for _ in range(NCORES)]
    for c in range(NCORES):
        rows = slice(c * P, (c + 1) * P)
        a_ioux = np.zeros((KX * 3, P, P), ml_dtypes.bfloat16)
        a_fx = np.zeros((KX, P, P), ml_dtypes.bfloat16)
        a_iouh = np.zeros((KC * 3, P, P), ml_dtypes.bfloat16)
        a_fh = np.zeros((KC, P, P), ml_dtypes.bfloat16)
        a_wrel = np.zeros((NR * KC, P, P), ml_dtypes.bfloat16)
        a_mf = np.zeros((NF * KC * 3, P, P), ml_dtypes.bfloat16)
        for g in range(3):
            gr = slice(g * MEM + c * P, g * MEM + (c + 1) * P)
            b_xi[c][g] = ioux_b[gr]
            b_iou[c][g] = iouh_b[gr]
            for k in range(KX):
                a_ioux[k * 3 + g] = _bf16(ioux_w[gr, k * P:(k + 1) * P].T)
            for j in range(KC):
                a_iouh[j * 3 + g] = _bf16(iouh_w[gr, j * P:(j + 1) * P].T)
            for r in fused_rels:
                fi = fslot_of[r]
                for j in range(KC):
                    a_mf[(fi * KC + j) * 3 + g] = _bf16(
                        Mfused[r][gr, j * P:(j + 1) * P].T)
        b_xf[c] = fx_b[rows]
        b_fh[c] = fh_b[rows]
        for k in range(KX):
            a_fx[k] = _bf16(fx_w[rows, k * P:(k + 1) * P].T)
        for j in range(KC):
            a_fh[j] = _bf16(fh_w[rows, j * P:(j + 1) * P].T)
            for ri, r in enumerate(used):
                a_wrel[ri * KC + j] = _bf16(Wrel[r][rows, j * P:(j + 1) * P].T)
        iouxs_h.append(a_ioux)
        fxs_h.append(a_fx)
        iouhs_h.append(a_iouh)
        fhs_h.append(a_fh)
        wrel_h.append(a_wrel)
        mf_h.append(np.ascontiguousarray(
            a_mf.transpose(1, 0, 2).reshape(P, -1)))

    # ---- build program ------------------------------------------------------
    nc = bass.Bass("TRN2", target_bir_lowering=False, debug=False,
                   num_devices=NCORES)
    d_xt = nc.dram_tensor("xt", [KX, P, N], BF16, kind="ExternalInput")
    d_iouxs = nc.dram_tensor("iouxs", [KX * 3, P, P], BF16, kind="ExternalInput")
    d_fxs = nc.dram_tensor("fxs", [KX, P, P], BF16, kind="ExternalInput")
    d_iouhs = nc.dram_tensor("iouhs", [KC * 3, P, P], BF16, kind="ExternalInput")
    d_fhs = nc.dram_tensor("fhs", [KC, P, P], BF16, kind="ExternalInput")
    d_wrel = nc.dram_tensor("wrel", [NR * KC, P, P], BF16, kind="ExternalInput")
    d_mf = nc.dram_tensor("mf", [P, NF * KC * 3 * P], BF16, kind="ExternalInput")
    d_bxi = nc.dram_tensor("b_xi", [3, P], F32, kind="ExternalInput")
    d_biou = nc.dram_tensor("b_iou", [3, P], F32, kind="ExternalInput")
    d_bxf = nc.dram_tensor("b_xf", [P], F32, kind="ExternalInput")
    d_bfh = nc.dram_tensor("b_fh", [P], F32, kind="ExternalInput")
    d_hout = nc.dram_tensor("hout", [P, N], F32, kind="ExternalOutput")

    ACT = mybir.ActivationFunctionType
    rg = [list(range(NCORES))]
    mf_loaded = [False]

    with tile.TileContext(nc, num_cores=NCORES) as tc:
        with (
            tc.tile_pool(name="const", bufs=1) as cpool,
            tc.tile_pool(name="state", bufs=1) as spool,
            tc.tile_pool(name="work", bufs=1) as wk,
            tc.tile_pool(name="psl", bufs=4, space="PSUM") as pp,
            tc.tile_pool(name="psg", bufs=2, space="PSUM") as pg,
            tc.tile_pool(name="dram", bufs=2, space="DRAM") as dp,
        ):
            def publish(src_tile, n, tag, land, loff):
                """AllGather my [P, n] bf16 slice into land[:, :, loff:loff+n]
                (slot j = core j's feature slice)."""
                gin = dp.tile([P, n], BF16, tag="gi" + tag)
                nc.sync.dma_start(gin[:], src_tile)
                gout = dp.tile([NCORES, P, n], BF16, tag="go" + tag,
                               addr_space="Shared")
                nc.gpsimd.collective_compute(
                    "AllGather", mybir.AluOpType.bypass,
                    ins=[gin.opt()], outs=[gout.opt()],
                    replica_groups=rg)
                nc.sync.dma_start(
                    land[:, :, loff:loff + n],
                    gout[:, :, :n].rearrange("k p n -> p k n"))

            # ---- constants (DMA in consumption order) ----------------------
            xt = cpool.tile([P, KX, N], BF16)
            nc.sync.dma_start(xt[:], d_xt.ap().rearrange("k p n -> p k n"))
            iouxs = cpool.tile([P, KX * 3, P], BF16)
            nc.sync.dma_start(iouxs[:], d_iouxs.ap().rearrange("s p m -> p s m"))
            fxs = cpool.tile([P, KX, P], BF16)
            nc.sync.dma_start(fxs[:], d_fxs.ap().rearrange("s p m -> p s m"))
            bxi = cpool.tile([P, 3], F32)
            nc.sync.dma_start(bxi[:], d_bxi.ap().rearrange("g p -> p g"))
            biou = cpool.tile([P, 3], F32)
            nc.sync.dma_start(biou[:], d_biou.ap().rearrange("g p -> p g"))
            bxf = cpool.tile([P, 1], F32)
            nc.sync.dma_start(bxf[:], d_bxf.ap().rearrange("(p one) -> p one", one=1))
            bfh = cpool.tile([P, 1], F32)
            nc.sync.dma_start(bfh[:], d_bfh.ap().rearrange("(p one) -> p one", one=1))
            fhs = cpool.tile([P, KC, P], BF16)
            nc.sync.dma_start(fhs[:], d_fhs.ap().rearrange("s p m -> p s m"))
            # The 11MB relation-weight stream rides the otherwise-idle gpsimd
            # queue in per-rel chunks so it never blocks the SP queue's
            # latency-critical publish DMAs or the ACT queue's activations.
            # A prefix is issued now; the rest after the leaf-AllGather
            # trigger so that trigger isn't stuck behind 60us of transfers.
            WPRE = min(20, NR)
            WSLOTS = max(NR * KC, NF * KC * 3)
            wrel_t = cpool.tile([P, WSLOTS, P], BF16)

            def wrel_dma(ri):
                nc.gpsimd.dma_start(
                    wrel_t[:, ri * KC:(ri + 1) * KC, :],
                    d_wrel.ap()[ri * KC:(ri + 1) * KC].rearrange(
                        "s p m -> p s m"))

            for ri in range(WPRE):
                wrel_dma(ri)
            iouhs = cpool.tile([P, KC * 3, P], BF16)

            # ---- state ------------------------------------------------------
            h_land = spool.tile([P, KC, NPAD], BF16)
            nc.vector.memset(h_land[:], 0.0)
            c_sl = spool.tile([P, NPAD], F32)
            nc.vector.memset(c_sl[:], 0.0)
            h_sl = spool.tile([P, N], F32)
            xi_f = spool.tile([P, 3, N], F32)
            xf_f = spool.tile([P, N], F32)

            # ---- xi / xf ----------------------------------------------------
            def xi_chunk(cc):
                ncc = min(P, N - cc)
                psi = pg.tile([P, 3, P], F32, tag="ps3")
                for g in range(3):
                    for k in range(KX):
                        nc.tensor.matmul(
                            psi[:, g, :ncc], iouxs[:, k * 3 + g, :],
                            xt[:, k, cc:cc + ncc],
                            start=(k == 0), stop=(k == KX - 1))
                for g in range(3):
                    nc.scalar.activation(
                        xi_f[:, g, cc:cc + ncc], psi[:, g, :ncc],
                        ACT.Identity, bias=bxi[:, g:g + 1])
                psf = pg.tile([P, K * NMAX], F32, tag="psf")
                for k in range(KX):
                    nc.tensor.matmul(
                        psf[:, :ncc], fxs[:, k, :], xt[:, k, cc:cc + ncc],
                        start=(k == 0), stop=(k == KX - 1))
                nc.scalar.activation(
                    xf_f[:, cc:cc + ncc], psf[:, :ncc], ACT.Identity,
                    bias=bxf[:, 0:1])

            # leaf columns first; the internal-node chunk is emitted after
            # the leaf publish so it fills PE idle time during the AllGather
            for cc in range(0, n_leaf, P):
                xi_chunk(cc)

            def gates(p0, n, iou_ps, fh_src, ccg, nch, big=False):
                NW = n_leaf if big else NMAX
                sfx = "L" if big else ""
                if iou_ps is not None:
                    tmp = wk.tile([P, 3, NW], F32, tag="gtmp" + sfx)
                    nc.vector.tensor_add(tmp[:, :, :n], iou_ps,
                                         xi_f[:, :, p0:p0 + n])
                ig = wk.tile([P, NW], F32, tag="ig" + sfx)
                og = wk.tile([P, NW], F32, tag="og" + sfx)
                ug = wk.tile([P, NW], F32, tag="ug" + sfx)
                srcs = ([tmp[:, g, :n] for g in range(3)] if iou_ps is not None
                        else [xi_f[:, g, p0:p0 + n] for g in range(3)])
                nc.scalar.activation(ig[:, :n], srcs[0], ACT.Sigmoid,
                                     bias=biou[:, 0:1])
                nc.scalar.activation(og[:, :n], srcs[1], ACT.Sigmoid,
                                     bias=biou[:, 1:2])
                nc.scalar.activation(ug[:, :n], srcs[2], ACT.Tanh,
                                     bias=biou[:, 2:3])
                cn = wk.tile([P, NW], F32, tag="cn" + sfx)
                nc.vector.tensor_mul(cn[:, :n], ig[:, :n], ug[:, :n])
                if fh_src is not None:
                    fsb = wk.tile([P, K * NMAX], F32, tag="fsb")
                    xfb = wk.tile([P, K * NMAX], F32, tag="xfb")
                    xfb_v = xfb[:, :nch].rearrange("p (n k) -> p n k", k=K)
                    for kk in range(K):
                        nc.vector.tensor_copy(
                            xfb_v[:, :, kk:kk + 1],
                            xf_f[:, p0:p0 + n].rearrange(
                                "p (n one) -> p n one", one=1))
                    nc.vector.tensor_add(fsb[:, :nch], fh_src, xfb[:, :nch])
                    nc.scalar.activation(fsb[:, :nch], fsb[:, :nch],
                                         ACT.Sigmoid, bias=bfh[:, 0:1])
                    nc.vector.tensor_mul(fsb[:, :nch], fsb[:, :nch],
                                         ccg[:, :nch])
                    fc = wk.tile([P, NMAX], F32, tag="fc")
                    nc.vector.tensor_reduce(
                        fc[:, :n],
                        fsb[:, :nch].rearrange("p (n k) -> p n k", k=K),
                        axis=mybir.AxisListType.X, op=mybir.AluOpType.add)
                    nc.vector.tensor_add(cn[:, :n], cn[:, :n], fc[:, :n])
                nc.vector.tensor_copy(c_sl[:, p0:p0 + n], cn[:, :n])
                tc_t = wk.tile([P, NW], F32, tag="tct" + sfx)
                nc.scalar.activation(tc_t[:, :n], cn[:, :n], ACT.Tanh)
                nc.vector.tensor_mul(h_sl[:, p0:p0 + n], og[:, :n], tc_t[:, :n])

            # ---- wave 0: leaves --------------------------------------------
            p0, p1 = waves[0]
            gates(p0, n_leaf, None, None, None, 0, big=True)
            hbL = wk.tile([P, n_leaf], BF16, tag="hbL")
            nc.vector.tensor_copy(hbL[:, :], h_sl[:, p0:p1])
            n1 = waves[1][1] - waves[1][0] if nwaves > 1 else 0
            split_w1 = False
            HLP = 0
            if nwaves > 1 and not fused[0] and len(wave_rels[0]) > 1 and n1 >= 64:
                cc1 = np.array([plan["child_col"][order[waves[1][0] + t]]
                                for t in range(n1)])
                half = n1 // 2
                a_real = cc1[:half][cc1[:half] != N]
                b_real = cc1[half:][cc1[half:] != N]
                if a_real.size and b_real.size:
                    HLP = int(a_real.max()) + 1
                    # two leaf AllGathers pay the ~19us collective cadence
                    # twice; measured slower than one big AG. Keep disabled.
                    split_w1 = False
            if split_w1:
                publish(hbL[:, :HLP], HLP, "La", h_land, 0)
                publish(hbL[:, HLP:n_leaf], n_leaf - HLP, "Lb", h_land, HLP)
            else:
                publish(hbL[:], n_leaf, "L", h_land, p0)
            # rest of the relation weights, behind the leaf-AG triggers
            for ri in range(WPRE, NR):
                wrel_dma(ri)
            # masks on SP and iouh/fused weights on ACT, positioned so they
            # block neither the leaf publish nor the xi/leaf activations
            nc.scalar.dma_start(iouhs[:],
                                d_iouhs.ap().rearrange("s p m -> p s m"))
            # xi for the internal-node columns fills the leaf-AG gap on PE
            for cc in range(n_leaf, N, P):
                xi_chunk(cc)

            # ---- internal waves --------------------------------------------
            for w in range(1, nwaves):
                wi = w - 1
                rels = wave_rels[wi]
                p0, p1 = waves[w]
                n = p1 - p0
                nch = n * K
                hch = wk.tile([P, KC, K * NMAX], BF16, tag="hch")
                ccg = wk.tile([P, K * NMAX], F32, tag="ccg")
                hsum_f = wk.tile([P, KC, NMAX], F32, tag="hsumf")
                hsum_b = wk.tile([P, KC, NMAX], BF16, tag="hsumb")
                psf = pg.tile([P, K * NMAX], F32, tag="psf")
                halves = ([(0, n // 2), (n // 2, n)]
                          if (w == 1 and split_w1) else [(0, n)])
                if rels and not fused[wi]:
                    zsl = wk.tile([P, NMAX], F32, tag="zsl")
                    zb = wk.tile([P, NMAX], BF16, tag="zb")
                for h0, h1 in halves:
                    c0, c1 = K * h0, K * h1
                    for (dst, src_c, ln) in wave_runs[wi]:
                        lo, hi = max(dst, c0), min(dst + ln, c1)
                        if lo >= hi:
                            continue
                        o = lo - dst
                        nc.vector.tensor_copy(
                            hch[:, :, lo:hi],
                            h_land[:, :, src_c + o:src_c + o + hi - lo])
                        nc.vector.tensor_copy(
                            ccg[:, lo:hi],
                            c_sl[:, src_c + o:src_c + o + hi - lo])
                    nc.vector.tensor_reduce(
                        hsum_f[:, :, h0:h1],
                        hch[:, :, c0:c1].rearrange("p k (n c) -> p k n c", c=K),
                        axis=mybir.AxisListType.X, op=mybir.AluOpType.add)
                    nc.vector.tensor_copy(hsum_b[:, :, h0:h1],
                                          hsum_f[:, :, h0:h1])
                    # fh matmuls don't depend on z: issued before the exchange
                    for j in range(KC):
                        nc.tensor.matmul(
                            psf[:, c0:c1], fhs[:, j, :], hch[:, j, c0:c1],
                            start=(j == 0), stop=(j == KC - 1))
                    if rels and not fused[wi]:
                        for r in rels:
                            sl = slot_of[r]
                            psl = pp.tile([P, P], F32, tag="psl")
                            for j in range(KC):
                                nc.tensor.matmul(
                                    psl[:, :h1 - h0],
                                    wrel_t[:, sl * KC + j, :],
                                    hsum_b[:, j, h0:h1],
                                    start=(j == 0), stop=(j == KC - 1))
                            # this relation owns disjoint columns: select
                            # them straight out of the psum tile
                            for t, ln in rel_runs[(wi, r)]:
                                if t >= h1 or t + ln <= h0:
                                    continue
                                a = max(t, h0)
                                b = min(t + ln, h1)
                                nc.vector.tensor_copy(
                                    zsl[:, a:b], psl[:, a - h0:b - h0])

                psi = pg.tile([P, 3, P], F32, tag="ps3")
                if fused[wi]:
                    # per-parent fused M_rel matmuls straight into iou psum.
                    # The relation-weight tile is dead once the last unfused
                    # wave finished, so the fused M slices land there (one
                    # DMA; Tile's WAR edge orders it after the last psl read)
                    if not mf_loaded[0]:
                        mf_loaded[0] = True
                        nc.sync.dma_start(
                            wrel_t[:, :NF * KC * 3, :],
                            d_mf.ap().rearrange("p (s m) -> p s m", m=P))
                    for r in rels:
                        fi = fslot_of[r]
                        for t in range(n):
                            if int(rel_ids[order[p0 + t]]) != r:
                                continue
                            for g in range(3):
                                for j in range(KC):
                                    nc.tensor.matmul(
                                        psi[:, g, t:t + 1],
                                        wrel_t[:, (fi * KC + j) * 3 + g, :],
                                        hsum_b[:, j, t:t + 1],
                                        start=(j == 0), stop=(j == KC - 1))
                elif rels:
                    nc.vector.tensor_copy(zb[:, :n], zsl[:, :n])
                    z_land = wk.tile([P, KC, NMAX], BF16, tag="zland")
                    publish(zb[:, :n], n, "z" + str(n), z_land, 0)
                    for g in range(3):
                        for j in range(KC):
                            nc.tensor.matmul(
                                psi[:, g, :n], iouhs[:, j * 3 + g, :],
                                z_land[:, j, :n],
                                start=(j == 0), stop=(j == KC - 1))
                else:
                    # all-identity wave: z == hsum
                    for g in range(3):
                        for j in range(KC):
                            nc.tensor.matmul(
                                psi[:, g, :n], iouhs[:, j * 3 + g, :],
                                hsum_b[:, j, :n],
                                start=(j == 0), stop=(j == KC - 1))

                gates(p0, n, psi[:, :, :n], psf[:, :nch], ccg, nch)
                if w < nwaves - 1:
                    hb = wk.tile([P, NMAX], BF16, tag="hb")
                    nc.vector.tensor_copy(hb[:, :n], h_sl[:, p0:p0 + n])
                    publish(hb[:, :n], n, "h" + str(n), h_land, p0)

            nc.sync.dma_start(d_hout.ap(), h_sl[:])

    _split_multi_waits(nc)

    in_maps = []
    for c in range(NCORES):
        in_maps.append({
            "xt": xt_h, "iouxs": iouxs_h[c], "fxs": fxs_h[c],
            "iouhs": iouhs_h[c], "fhs": fhs_h[c], "wrel": wrel_h[c],
            "mf": mf_h[c], "b_xi": b_xi[c],
            "b_iou": b_iou[c], "b_xf": b_xf[c], "b_fh": b_fh[c],
        })
    kernel._nc = nc
    kernel._in_maps = in_maps
    res = run_bass_kernel_spmd(nc, in_maps, list(range(NCORES)))
    hT = np.concatenate([res.results[c]["hout"] for c in range(NCORES)], 0)
    out = np.empty((N, MEM), np.float32)
    for node in range(N):
        out[node] = hT[:, col_of[node]]
    return out
